# revision 29
# baseline (speedup 1.0000x reference)
"""MixtureLinear Trainium2 kernel.

Computes, for B=256, IN=1024, OUT=1024, RANK=16:
    out[b,o] = sum_i input[b,i] * sum_r weight[o,i,r] * coef[b,r]
             + sum_r bias[o,r] * coef[b,r]

Strategy (8 NeuronCores, tensor-parallel on OUT):
  - Core c owns OUT rows [128c, 128c+128). It reads only its weight shard
    (1/8 of the 64MB weight tensor), input/coef replicated.
  - Stage 1 (PE): proj[b,(o,r)] = inputT.T @ W2 where W2[i, o*16+r] =
    weight[o,i,r]; K=IN accumulated over 8 psum matmuls per 512-column
    chunk (one psum bank, 32 o's x 16 r's per chunk).
  - Stage 2 (DVE): out[b,o] = sum_r proj[b,(o,r)] * coef[b,r] via a
    broadcast-AP multiply + strided reduce over the innermost rank axis.
  - Bias: one tiny K=16 matmul per b-chunk: coefT.T @ biasT -> psum,
    added in the final DVE add before the output DMA.

Matmul dtype is selectable via MIXL_DT (float16 default; bfloat16 /
float32r / float32 supported). Host pre-casts and pre-transposes shards;
stage-2 and all accumulation stay fp32; the output ships fp16 and is
upcast on host (~1.5e-4 extra rounding vs the 2e-2 gate).

Two implementations: "raw" (the original baseline, ~41-45us) and "raw2"
(default, ~34us) - see build_nc_raw2's docstring for the trace-derived
design notes (single-queue whole-transfer DMA, garbage-data HAM warm-up,
split last chunk, early b0 finalize, queue-warming throwaway read).
"""

import os
import sys
from contextlib import ExitStack

sys.path.insert(0, "/opt/trn_rl_repo")

import numpy as np
import ml_dtypes

import concourse.bass as bass
import concourse.tile as tile
from concourse import bacc, mybir
from concourse.bass_utils import run_bass_kernel_spmd

B, IN, OUT, RANK = 256, 1024, 1024, 16
NCORES = 8
OUTL = OUT // NCORES        # 128 out rows per core
P = 128                     # partitions
NB = B // P                 # 2 batch chunks
NK = IN // P                # 8 contraction chunks
CH = 512                    # psum chunk: one fp32 bank
NCH = OUTL * RANK // CH     # 4 column chunks per core
OCH = CH // RANK            # 32 o's per chunk

DT_NAME = os.environ.get("MIXL_DT", "float16")
# Weight-only dtype for raw2 (moving operand of the big matmuls). fp8e3
# (E3M4) halves the dominant DMA stream; weights are pre-scaled by
# W_SCALE on host (pushes values out of the subnormal range) and the
# inverse is folded into the stage-2 coef. Measured L2 err 1.34e-2 vs
# the 2e-2 gate on the fixed harness seed (fp16: 3.6e-4).
# NOTE: fp8 weights are DEAD on this HW: PE matmul with fp16 lhsT and
# fp8e3 rhs dies with NRT_EXEC_UNIT_UNRECOVERABLE (mixed operand dtypes
# unsupported), and both-operands-e3m4 measures L2=1.94e-2 vs the 2e-2
# gate - no margin. Keep float16.
WDT_NAME = os.environ.get("MIXL_WDT", "float16")
W_SCALE = float(os.environ.get("MIXL_WSCALE", "16"))
IMPL = os.environ.get("MIXL_IMPL", "raw2")
# PE keep-alive dummies before the first real matmul (raw2): long = N=512
# (427-512ns at slow clock), run before the s_x wait; short = N=128
# (54-68ns), run between s_x and the s_w0 wait to adapt to DMA progress.
N_WARM_LONG = int(os.environ.get("MIXL_WARM_LONG", "8"))
N_WARM_SHORT = int(os.environ.get("MIXL_WARM_SHORT", "55"))

_DT_MAP = {
    "float16": (mybir.dt.float16, np.float16),
    "bfloat16": (mybir.dt.bfloat16, ml_dtypes.bfloat16),
    "float32r": (mybir.dt.float32r, np.float32),
    "float32": (mybir.dt.float32, np.float32),
    "float8e3": (mybir.dt.float8e3, ml_dtypes.float8_e3m4),
    "float8e4": (mybir.dt.float8e4, ml_dtypes.float8_e4m3),
}


class _NoBarrierBlock(bass.BassBlock):
    """BassBlock without the exit drain + all-engine barrier.

    The NRT epilogue (per-engine semaphore-zero storm + exit rendezvous
    chain, ~7us total) runs after each engine's stream ends. With the
    stock barrier, every engine waits for the slowest one before starting
    its epilogue share; without it, early-finishing engines overlap their
    epilogue with the critical-path tail. Output completion is still
    guaranteed: gpsimd's terminal s_out wait orders NEFF completion after
    the output DMAs.
    """

    def __exit__(self, exc_type, exc_val, exc_tb):
        if exc_type is not None:
            return
        for engine, last_body in self.last_body.items():
            with self.bass.body(
                last_body, parent=self.bass.cur_bb, allow_existing_parent=True
            ):
                engine.br(self.end_bb)
        self.bass.switch_bb(self.end_bb)


def build_nc_raw(dt_name=DT_NAME):
    """Raw-Bass (manual Block + semaphores) implementation.

    Design notes (all HW-measured on this container):
    - The NRT execution envelope (entry rendezvous chain + per-engine
      instruction-table loads at the front; per-engine semaphore-zero
      storm + exit rendezvous at the back) costs ~14us on an empty
      kernel. The exit part runs after each ENGINE's stream ends, so the
      Block-end all-engine barrier is deliberately skipped (engines that
      finish early overlap their epilogue with the critical-path tail).
    - A single DMA transfer only sustains ~100-200 GB/s; aggregate tops
      out ~430 GB/s, and the SDMA engines round-robin across everything
      queued at packet granularity. So loads go out in consumption order
      as ~10 chunks with a sliding window of 3 in flight, with tiny
      first chunks so the first matmul can start ASAP.
    - 8 dummy matmuls on memset data warm the PE HAM clock (1.2->2.4GHz
      after ~3.4us of sustained activity) while the first loads land.
    - LDWEIGHTS is emitted separately from a non-self-loading Matmult
      (inst.ldweights=False) so weight loads pipeline into the PE's
      background buffer; fused matmuls measured ~600ns vs ~380ns split.
    """
    dt, _ = _DT_MAP[dt_name]
    f32 = mybir.dt.float32
    f16 = mybir.dt.float16
    nc = bacc.Bacc("TRN2", target_bir_lowering=False, debug=False)

    xT = nc.declare_dram_parameter("xT", [IN, B], dt, isOutput=False)
    w2 = nc.declare_dram_parameter("w2", [NCH, P, NK * CH], dt, isOutput=False)
    coef = nc.declare_dram_parameter("coef", [B, RANK], f32, isOutput=False)
    coefT = nc.declare_dram_parameter("coefT", [RANK, B], dt, isOutput=False)
    biasT = nc.declare_dram_parameter("biasT", [RANK, OUTL], dt, isOutput=False)
    out = nc.declare_dram_parameter("out", [B, OUTL], f32, isOutput=True)

    w2v = w2.rearrange("n p (k c) -> n p k c", c=CH)
    xTv = xT.rearrange("(k p) b -> p k b", p=P)
    coefv = coef.rearrange("(nb p) r -> p nb r", p=P)

    with ExitStack() as ctx:
        sb = lambda shape, d, name: ctx.enter_context(
            nc.sbuf_tensor(name, shape, d))
        xT_t = sb([P, NK, B], dt, "xT_t")
        wts = [sb([P, NK, CH], dt, f"wt{n}") for n in range(NCH)]
        coef_t = sb([P, NB, RANK], f32, "coef_t")
        coefT_t = sb([RANK, B], dt, "coefT_t")
        biasT_t = sb([RANK, OUTL], dt, "biasT_t")
        warm_t = sb([P, CH], dt, "warm_t")
        tmps = [sb([P, OCH, RANK], f16, f"tmp{i}") for i in range(2)]
        out_sb = [sb([P, OUTL], f32, f"osum{b}") for b in range(NB)]
        outf = [sb([P, OUTL], f32, f"outf{b}") for b in range(NB)]
        pss = [ctx.enter_context(nc.psum_tensor(f"ps{g}", [P, CH], f32))
               for g in range(8)]

        # One semaphore per DMA: +16 increments from different transfers
        # interleave (per-SDMA-engine +1s), so aggregate thresholds on a
        # shared sem do not prove any single transfer completed.
        nsem = lambda name: ctx.enter_context(nc.semaphore(name))
        s_x0 = nsem("s_x0")        # xT k=0 (64KB)
        s_w00 = nsem("s_w00")      # w n0 k=0 (128KB)
        s_x13 = nsem("s_x13")      # xT k=1..3
        s_w013 = nsem("s_w013")    # w n0 k=1..3
        s_x47 = nsem("s_x47")      # xT k=4..7
        s_w047 = nsem("s_w047")    # w n0 k=4..7
        s_wn = [nsem(f"s_wn{n}") for n in range(1, NCH)]   # w1..w3 (1MB)
        s_gc = nsem("s_gc")        # coefT
        s_gb = nsem("s_gb")        # biasT
        s_gf = nsem("s_gf")        # coef (fp32)
        s_warm = nsem("s_warm")    # warm-up tile memset
        s_pe = nsem("s_pe")        # psum groups done
        s_dvm = nsem("s_dvm")      # psum mults done
        s_red = nsem("s_red")      # reduces done
        s_dve = nsem("s_dve")      # outf ready
        s_out = nsem("s_out")      # output DMA done

        with _NoBarrierBlock(nc, f"block_{nc.next_id()}") as block:

            @block.sync
            def _(sync):
                xfers = [
                    (xT_t[:, 0:1, :], xTv[:, 0:1, :], s_x0),
                    (wts[0][:, 0:1, :], w2v[0][:, 0:1, :], s_w00),
                    (xT_t[:, 1:4, :], xTv[:, 1:4, :], s_x13),
                    (wts[0][:, 1:4, :], w2v[0][:, 1:4, :], s_w013),
                    (xT_t[:, 4:, :], xTv[:, 4:, :], s_x47),
                    (wts[0][:, 4:, :], w2v[0][:, 4:, :], s_w047),
                ] + [(wts[n][:], w2v[n], s_wn[n - 1]) for n in range(1, NCH)]
                for i, (dst, srcv, sem) in enumerate(xfers):
                    if i >= 3:
                        sync.wait_ge(xfers[i - 3][2], 16)
                    sync.dma_start(dst, srcv).then_inc(sem, 16)

            @block.scalar
            def _(scalar):
                # Output DMAs on the (otherwise idle) ACT ring.
                for b in range(NB):
                    scalar.wait_ge(s_dve, b + 1)
                    scalar.dma_start(out[b * P:(b + 1) * P, :],
                                     outf[b][:]).then_inc(s_out, 16)

            @block.gpsimd
            def _(gpsimd):
                gpsimd.memset(warm_t[:], 0.25).then_inc(s_warm, 1)
                gpsimd.dma_start(coef_t[:], coefv).then_inc(s_gf, 16)
                gpsimd.dma_start(coefT_t[:], coefT[:]).then_inc(s_gc, 16)
                gpsimd.dma_start(biasT_t[:], biasT[:]).then_inc(s_gb, 16)
                # Terminal waiter: holds the Pool stream until outputs are
                # in DRAM, so NEFF completion implies outputs landed.
                gpsimd.wait_ge(s_out, 32)

            @block.tensor
            def _(pe):
                # HAM warm-up: ~8 dummy matmuls on memset data while the
                # first real chunks are still in flight.
                pe.wait_ge(s_warm, 1)
                for _i in range(8):
                    nc.tensor.matmul(pss[2][:], lhsT=warm_t[:, 0:P],
                                     rhs=warm_t[:], start=True, stop=True)
                for n in range(NCH):
                    bank = (2 * n) % 6
                    for k in range(NK):
                        if n == 0:
                            if k == 0:
                                pe.wait_ge(s_x0, 16)
                                pe.wait_ge(s_w00, 16)
                            elif k == 1:
                                pe.wait_ge(s_x13, 16)
                                pe.wait_ge(s_w013, 16)
                            elif k == 4:
                                pe.wait_ge(s_x47, 16)
                                pe.wait_ge(s_w047, 16)
                        elif k == 0:
                            pe.wait_ge(s_wn[n - 1], 16)
                        if n == 3 and k == 0:
                            # banks 0/1 reused: n0 multiplies must be done
                            pe.wait_ge(s_dvm, 2)
                        for b in range(NB):
                            # split LDWEIGHTS + non-self-loading matmul
                            nc.tensor.ldweights(xT_t[:, k, b * P:(b + 1) * P])
                            mm = nc.tensor.matmul(
                                pss[bank + b][:],
                                lhsT=xT_t[:, k, b * P:(b + 1) * P],
                                rhs=wts[n][:, k, :],
                                start=(k == 0),
                                stop=(k == NK - 1),
                            )
                            mm.ins.ldweights = False
                            if k == NK - 1:
                                mm.then_inc(s_pe, 1)
                    if n == 0:
                        # Bias matmuls into dedicated banks 6/7, slotted here
                        # so their input DMAs are long done and the PE stream
                        # never stalls on them.
                        pe.wait_ge(s_gc, 16)
                        pe.wait_ge(s_gb, 16)
                        for b in range(NB):
                            nc.tensor.matmul(
                                pss[6 + b][:, 0:OUTL],
                                lhsT=coefT_t[:, b * P:(b + 1) * P],
                                rhs=biasT_t[:],
                                start=True, stop=True,
                            ).then_inc(s_pe, 1)

            @block.vector
            def _(vector):
                vector.wait_ge(s_gf, 16)
                # s_pe increment order: n0b0=1 n0b1=2 bias0=3 bias1=4
                # n1b0=5 n1b1=6 n2b0=7 n2b1=8 n3b0=9 n3b1=10
                pe_val = {0: (1, 2), 1: (5, 6), 2: (7, 8), 3: (9, 10)}
                g = 0
                for n in range(NCH):
                    bank = (2 * n) % 6
                    for b in range(NB):
                        g += 1
                        vector.wait_ge(s_pe, pe_val[n][b])
                        coef_b = coef_t[:, b, :].rearrange(
                            "p (one r) -> p one r", one=1)
                        tmp = tmps[g % 2]
                        nc.vector.tensor_mul(
                            tmp[:],
                            pss[bank + b][:].rearrange("p (o r) -> p o r", r=RANK),
                            coef_b.to_broadcast((P, OCH, RANK)),
                        ).then_inc(s_dvm, 1)
                        vector.wait_ge(s_dvm, g)
                        nc.vector.tensor_reduce(
                            out_sb[b][:, n * OCH:(n + 1) * OCH],
                            tmp[:],
                            axis=mybir.AxisListType.X,
                            op=mybir.AluOpType.add,
                        ).then_inc(s_red, 1)
                for b in range(NB):
                    vector.wait_ge(s_pe, 3 + b)
                    # all four reduces of this b-chunk (g = b+1, b+3, b+5, b+7)
                    vector.wait_ge(s_red, NB * NCH - NB + b + 1)
                    nc.vector.tensor_add(
                        outf[b][:], out_sb[b][:], pss[6 + b][:, 0:OUTL]
                    ).then_inc(s_dve, 1)

    nc.compile()
    return nc


def build_nc_raw2(dt_name=DT_NAME, wdt_name=WDT_NAME):
    """v4: whole-tensor single-queue DMA, garbage warmup, lean tail.

    Trace findings this encodes (HW-measured on this container):
    - DMA packet = per-partition contiguous run; 8KB runs hit 350-440
      GB/s through one HWDGE queue, small runs dispatch far slower. The
      two HWDGE queues share one ~430 GB/s aggregate, so all big loads
      go on ONE queue (sync) in consumption order: xT (4KB runs), then
      w0..w3 (8KB runs), sliding window 4 (ring-depth guard; the waits
      are free since a queue completes in order).
    - First-packet latency is ~2.3us after doorbell, so data can't
      arrive before ~9us no matter what; the PE runs warm-up dummies on
      garbage SBUF from stream start (~6.5us) to hold the HAM clock
      ramp, with a wait-for-xT between long and short dummies so the
      dummy count adapts to actual DMA progress.
    - HAM clock ramps (1.0/1.2 -> 2.0/2.4GHz) after ~3.4us of gapless
      PE activity and drops after ~2us idle; run-to-run DVFS varies
      +-20%, so judge by gaps not absolute ns.
    - DVE (stage 2) is the tail: tensor_reduce and PSUM-operand
      tensor_tensor are capped at 1 elem/cycle/lane, total DVE ~= PE
      time, so the DVE runs ~1us past the last matmul. The last column
      chunk is split into two 256-col halves on banks 0/1 then 2/3
      (matmul psum dst stays at bank offset 0) to shrink the last
      piece; in-order execution makes reduce-after-mult safe without
      semaphore waits (s_dvm increments are kept only for the PE's
      bank-reuse gates).
    - Output stores pay the same ~1.5-2us queue latency; a throwaway
      store to a scratch DRAM tensor is issued when n3 begins to warm
      the scalar queue, then out[b0] / out[b1] ship as two transfers.
    """
    dt, _ = _DT_MAP[dt_name]
    wdt, _ = _DT_MAP[wdt_name]
    f32 = mybir.dt.float32
    f16 = mybir.dt.float16
    nc = bacc.Bacc("TRN2", target_bir_lowering=False, debug=False)

    xT = nc.declare_dram_parameter("xT", [P, NK * B], dt, isOutput=False)
    w2 = nc.declare_dram_parameter("w2", [NCH, P, NK * CH], wdt, isOutput=False)
    coef = nc.declare_dram_parameter("coef", [B, RANK], f32, isOutput=False)
    coefT = nc.declare_dram_parameter("coefT", [RANK, B], dt, isOutput=False)
    biasT = nc.declare_dram_parameter("biasT", [RANK, OUTL], dt, isOutput=False)
    # fp16 output: halves the output-store bytes on the critical tail;
    # the host upcasts to fp32 (adds ~1.5e-4 rounding vs the 2e-2 gate).
    out = nc.declare_dram_parameter("out", [B, OUTL], f16, isOutput=True)

    xTv = xT.rearrange("p (k b) -> p k b", b=B)
    coefv = coef.rearrange("(nb p) r -> p nb r", p=P)

    with ExitStack() as ctx:
        sb = lambda shape, d, name: ctx.enter_context(
            nc.sbuf_tensor(name, shape, d))
        xT_t = sb([P, NK, B], dt, "xT_t")
        wts = [sb([P, NK, CH], wdt, f"wt{n}") for n in range(NCH)]
        coef_t = sb([P, NB, RANK], f32, "coef_t")
        coefT_t = sb([RANK, B], dt, "coefT_t")
        biasT_t = sb([RANK, OUTL], dt, "biasT_t")
        warm_t = sb([P, CH], dt, "warm_t")
        tmps = [sb([P, OCH, RANK], f16, f"tmp{i}") for i in range(2)]
        out_sb = [sb([P, OUTL], f32, f"osum{b}") for b in range(NB)]
        outf = [sb([P, OUTL], f16, f"outf{b}") for b in range(NB)]
        pss = [ctx.enter_context(nc.psum_tensor(f"ps{g}", [P, CH], f32))
               for g in range(8)]

        nsem = lambda name: ctx.enter_context(nc.semaphore(name))
        s_x = nsem("s_x")          # xT
        s_w = [nsem(f"s_w{n}") for n in range(NCH)]
        s_gcb = nsem("s_gcb")      # coefT + biasT (waited jointly)
        s_gf = nsem("s_gf")        # coef (fp32)
        s_pe = nsem("s_pe")        # psum groups done
        s_dvm = nsem("s_dvm")      # psum mults done (PE bank-reuse gate)
        s_dve = nsem("s_dve")      # outf pieces ready
        s_out = nsem("s_out")      # output DMA done

        with _NoBarrierBlock(nc, f"block_{nc.next_id()}") as block:

            @block.sync
            def _(sync):
                # One queue, consumption order, whole transfers (max
                # run size = max dispatch rate; k-half splits measured
                # SLOWER overall). The HWDGE round-robins two active
                # transfers byte-fairly, so w0 would otherwise finish
                # ~half-of-w1 late: the explicit wait after w0 keeps w1
                # out of the queue until w0 completes, trading a ~0.7us
                # dispatch bubble (absorbed by w1-w3's slack) for a
                # ~2.5us earlier n0 start.
                # Keep the queue full at all times: a mid-stream drain
                # (wait before the next issue) costs a ~2.5-3us restart
                # bubble, far worse than the ~2-transfer round-robin
                # completion lag. w0 leads (the lead transfer drains at
                # full share; its partner x is half-sized), w1-w3 queue
                # behind under a ring-depth window of 4.
                xfers = [
                    (xT_t[:], xTv[:], s_x),
                    (wts[0][:], w2[0], s_w[0]),
                    (wts[1][:], w2[1], s_w[1]),
                    (wts[2][:], w2[2], s_w[2]),
                    (wts[3][:], w2[3], s_w[3]),
                ]
                for i, (dst, srcv, sem) in enumerate(xfers):
                    if i >= 4:
                        sync.wait_ge(xfers[i - 4][2], 16)
                    sync.dma_start(dst, srcv).then_inc(sem, 16)

            @block.scalar
            def _(scalar):
                # Tiny loads early; scratch store to warm the queue when
                # n3 starts; then the real output stores.
                scalar.dma_start(coef_t[:], coefv).then_inc(s_gf, 16)
                scalar.dma_start(coefT_t[:], coefT[:]).then_inc(s_gcb, 16)
                scalar.dma_start(biasT_t[:], biasT[:]).then_inc(s_gcb, 16)
                # Throwaway reads: warm the queue's descriptor pipeline
                # just before the real output stores (HWDGE warmth decays,
                # so fire close to use). (walrus codegen requires DMAs to
                # carry a completion sem.)
                scalar.wait_ge(s_pe, 10)
                scalar.dma_start(warm_t[:, 0:64],
                                 xTv[:, 0, 0:64]).then_inc(s_out, 16)
                scalar.wait_ge(s_pe, 12)
                scalar.dma_start(warm_t[:, 64:128],
                                 xTv[:, 0, 64:128]).then_inc(s_out, 16)
                scalar.wait_ge(s_dve, 1)
                scalar.dma_start(out[0:P, :], outf[0][:]).then_inc(s_out, 16)
                scalar.wait_ge(s_dve, 2)
                scalar.dma_start(out[P:2 * P, :], outf[1][:]).then_inc(
                    s_out, 16)

            @block.gpsimd
            def _(gpsimd):
                gpsimd.memset(warm_t[:], 0.25)
                # Terminal waiter: NEFF completion implies outputs landed.
                # 64 = two warm reads + two output stores (16 each).
                gpsimd.wait_ge(s_out, 64)

            @block.tensor
            def _(pe):
                # Garbage-data dummies from stream start hold the HAM
                # ramp; the s_x wait adapts the count to DMA progress.
                for _i in range(N_WARM_LONG):
                    nc.tensor.matmul(pss[2][:], lhsT=warm_t[:, 0:P],
                                     rhs=warm_t[:], start=True, stop=True)
                pe.wait_ge(s_x, 16)
                for _i in range(N_WARM_SHORT):
                    nc.tensor.matmul(pss[2][:, 0:P], lhsT=warm_t[:, 0:P],
                                     rhs=warm_t[:, 0:P], start=True, stop=True)

                def mm(n, k, b, inc=False):
                    nc.tensor.ldweights(xT_t[:, k, b * P:(b + 1) * P])
                    m = nc.tensor.matmul(
                        pss[(2 * n) % 6 + b][:],
                        lhsT=xT_t[:, k, b * P:(b + 1) * P],
                        rhs=wts[n][:, k, :],
                        start=(k == 0),
                        stop=(k == NK - 1),
                    )
                    m.ins.ldweights = False
                    if inc:
                        m.then_inc(s_pe, 1)

                # n0: k-major (b inner).          s_pe: 1, 2
                pe.wait_ge(s_w[0], 16)
                for k in range(NK):
                    for b in range(NB):
                        mm(0, k, b, inc=(k == NK - 1))
                # Bias matmuls into banks 6/7.    s_pe: 3, 4
                pe.wait_ge(s_gcb, 32)
                for b in range(NB):
                    nc.tensor.matmul(
                        pss[6 + b][:, 0:OUTL],
                        lhsT=coefT_t[:, b * P:(b + 1) * P],
                        rhs=biasT_t[:],
                        start=True, stop=True,
                    ).then_inc(s_pe, 1)
                # n1, n2.                         s_pe: 5,6 / 7,8
                for n in (1, 2):
                    pe.wait_ge(s_w[n], 16)
                    for k in range(NK):
                        for b in range(NB):
                            mm(n, k, b, inc=(k == NK - 1))
                # n3: two 256-col halves at bank offset 0 — h0 on banks
                # 0/1 (needs n0 mults done), h1 on banks 2/3 (needs n1
                # mults done). s_pe: 9,10 (h0 b0,b1) / 11,12 (h1)
                pe.wait_ge(s_w[3], 16)
                HC = CH // 2
                for h in range(2):
                    pe.wait_ge(s_dvm, 2 * (h + 1))
                    for b in range(NB):
                        for k in range(NK):
                            nc.tensor.ldweights(
                                xT_t[:, k, b * P:(b + 1) * P])
                            m = nc.tensor.matmul(
                                pss[2 * h + b][:, 0:HC],
                                lhsT=xT_t[:, k, b * P:(b + 1) * P],
                                rhs=wts[3][:, k, h * HC:(h + 1) * HC],
                                start=(k == 0),
                                stop=(k == NK - 1),
                            )
                            m.ins.ldweights = False
                            if k == NK - 1:
                                m.then_inc(s_pe, 1)

            @block.vector
            def _(vector):
                vector.wait_ge(s_gf, 16)

                def stage2(bank, b, pe_th, o0, no, c0=0):
                    # psum[bank] cols [c0, c0+no*RANK) * coef_b, rank-
                    # reduced into out_sb[b][:, o0:o0+no]. The reduce
                    # needs no wait: the engine is in-order and the mult
                    # precedes it.
                    vector.wait_ge(s_pe, pe_th)
                    coef_b = coef_t[:, b, :].rearrange(
                        "p (one r) -> p one r", one=1)
                    tmp = tmps[pe_th % 2]
                    nc.vector.tensor_mul(
                        tmp[:, 0:no, :],
                        pss[bank][:, c0:c0 + no * RANK].rearrange(
                            "p (o r) -> p o r", r=RANK),
                        coef_b.to_broadcast((P, no, RANK)),
                    ).then_inc(s_dvm, 1)
                    nc.vector.tensor_reduce(
                        out_sb[b][:, o0:o0 + no],
                        tmp[:, 0:no, :],
                        axis=mybir.AxisListType.X,
                        op=mybir.AluOpType.add,
                    )

                pe_val = {0: (1, 2), 1: (5, 6), 2: (7, 8)}
                for n in range(NCH - 1):
                    for b in range(NB):
                        stage2((2 * n) % 6 + b, b, pe_val[n][b],
                               n * OCH, OCH)
                HO = OCH // 2
                stage2(0, 0, 9, 3 * OCH, HO)
                stage2(1, 1, 10, 3 * OCH, HO)
                stage2(2, 0, 11, 3 * OCH + HO, HO)
                # b0 finalize: all b0 reduces done (program order), bias
                # b0 in psum bank 6 (s_pe>=3 long satisfied).
                vector.wait_ge(s_pe, 3)
                nc.vector.tensor_add(
                    outf[0][:], out_sb[0][:], pss[6][:, 0:OUTL]
                ).then_inc(s_dve, 1)
                stage2(3, 1, 12, 3 * OCH + HO, HO)
                vector.wait_ge(s_pe, 4)
                nc.vector.tensor_add(
                    outf[1][:], out_sb[1][:], pss[7][:, 0:OUTL]
                ).then_inc(s_dve, 1)

    nc.compile()
    return nc


def build_nc_tile(dt_name=DT_NAME):
    dt, _ = _DT_MAP[dt_name]
    f32 = mybir.dt.float32
    # Bacc (not raw Bass): its compile() runs generate_event_semaphores,
    # which splits multi-wait sync_info into EventSemaphore prefixes —
    # walrus accepts at most one wait per regular instruction.
    nc = bacc.Bacc("TRN2", target_bir_lowering=False, debug=False)

    xT = nc.declare_dram_parameter("xT", [IN, B], dt, isOutput=False)
    # w2[n, p, k*CH+c] = W2[k*128+p, n*CH+c]: pre-swizzled on host so each
    # SBUF partition's data is one contiguous 8KB run in DRAM (full-rate DMA).
    w2 = nc.declare_dram_parameter("w2", [NCH, P, NK * CH], dt, isOutput=False)
    coef = nc.declare_dram_parameter("coef", [B, RANK], f32, isOutput=False)
    coefT = nc.declare_dram_parameter("coefT", [RANK, B], dt, isOutput=False)
    biasT = nc.declare_dram_parameter("biasT", [RANK, OUTL], dt, isOutput=False)
    out = nc.declare_dram_parameter("out", [B, OUTL], f32, isOutput=True)

    with tile.TileContext(nc) as tc, ExitStack() as ctx:
        cpool = ctx.enter_context(tc.tile_pool(name="const", bufs=1))
        wpool = ctx.enter_context(tc.tile_pool(name="w", bufs=NCH))
        ppool = ctx.enter_context(tc.tile_pool(name="proj", bufs=6, space="PSUM"))
        bpool = ctx.enter_context(tc.tile_pool(name="biasps", bufs=2, space="PSUM"))
        spool = ctx.enter_context(tc.tile_pool(name="stage2", bufs=4))
        opool = ctx.enter_context(tc.tile_pool(name="outp", bufs=2))

        # Weight tiles for every n-chunk (issued first; n=0 split so the
        # first matmuls can start after only 256KB has landed).
        wts = [wpool.tile([P, NK, CH], dt, tag="w", name=f"wt{n}")
               for n in range(NCH)]
        w2v = w2.rearrange("n p (k c) -> n p k c", c=CH)
        nc.sync.dma_start(wts[0][:, 0:2, :], w2v[0][:, 0:2, :])
        # Full inputT, split in halves (first matmuls need only low k).
        xT_t = cpool.tile([P, NK, B], dt, tag="xT")
        xTv = xT.rearrange("(k p) b -> p k b", p=P)
        nc.sync.dma_start(xT_t[:, 0:NK // 2, :], xTv[:, 0:NK // 2, :])
        nc.sync.dma_start(wts[0][:, 2:NK, :], w2v[0][:, 2:NK, :])
        nc.sync.dma_start(xT_t[:, NK // 2:, :], xTv[:, NK // 2:, :])
        for n in range(1, NCH):
            nc.sync.dma_start(wts[n][:], w2[n].rearrange("p (k c) -> p k c", c=CH))
        coef_t = cpool.tile([P, NB, RANK], f32, tag="coef")
        nc.sync.dma_start(coef_t[:], coef.rearrange("(nb p) r -> p nb r", p=P))
        coefT_t = cpool.tile([RANK, B], dt, tag="coefT")
        nc.sync.dma_start(coefT_t[:], coefT[:])
        biasT_t = cpool.tile([RANK, OUTL], dt, tag="biasT")
        nc.sync.dma_start(biasT_t[:], biasT[:])

        # Bias term: out_bias[b,o] = sum_r coef[b,r] * bias[o,r]
        bias_ps = []
        for b in range(NB):
            bp = bpool.tile([P, OUTL], f32, tag="bias")
            nc.tensor.matmul(
                bp[:], lhsT=coefT_t[:, b * P:(b + 1) * P], rhs=biasT_t[:],
                start=True, stop=True,
            )
            bias_ps.append(bp)

        out_sb = [
            opool.tile([P, OUTL], f32, tag="osum", name=f"osum{b}")
            for b in range(NB)
        ]

        for n in range(NCH):
            pss = [
                ppool.tile([P, CH], f32, tag="proj", name=f"proj{n}_{b}")
                for b in range(NB)
            ]
            wt = wts[n]
            for k in range(NK):
                for b in range(NB):
                    nc.tensor.matmul(
                        pss[b][:],
                        lhsT=xT_t[:, k, b * P:(b + 1) * P],
                        rhs=wt[:, k, :],
                        start=(k == 0),
                        stop=(k == NK - 1),
                    )
            # Rank contraction: multiply by per-(b,r) coef, reduce over r.
            for b in range(NB):
                tmp = spool.tile([P, CH], f32, tag="tmp")
                coef_b = coef_t[:, b, :].rearrange("p (one r) -> p one r", one=1)
                nc.vector.tensor_mul(
                    tmp[:].rearrange("p (o r) -> p o r", r=RANK),
                    pss[b][:].rearrange("p (o r) -> p o r", r=RANK),
                    coef_b.to_broadcast((P, OCH, RANK)),
                )
                nc.vector.tensor_reduce(
                    out_sb[b][:, n * OCH:(n + 1) * OCH],
                    tmp[:].rearrange("p (o r) -> p o r", r=RANK),
                    axis=mybir.AxisListType.X,
                    op=mybir.AluOpType.add,
                )

        for b in range(NB):
            outf = opool.tile([P, OUTL], f32, tag="outf")
            nc.vector.tensor_add(outf[:], out_sb[b][:], bias_ps[b][:])
            nc.sync.dma_start(out[b * P:(b + 1) * P, :], outf[:])

    nc.compile()
    return nc


def prepare_in_maps(input, coef, weight, bias, dt_name=DT_NAME, impl=IMPL,
                    wdt_name=WDT_NAME):
    _, npdt = _DT_MAP[dt_name]
    if impl == "raw2":
        _, npwdt = _DT_MAP[wdt_name]
        wscale = W_SCALE if wdt_name.startswith("float8") else 1.0
        # [P, NK, B]: per-partition 4KB contiguous runs -> big DMA packets.
        xT = np.ascontiguousarray(
            input.T.reshape(NK, P, B).transpose(1, 0, 2).reshape(P, NK * B)
        ).astype(npdt)
    else:
        npwdt = npdt
        wscale = 1.0
        xT = np.ascontiguousarray(input.T).astype(npdt)      # (IN, B)
    coefT = np.ascontiguousarray(coef.T).astype(npdt)        # (RANK, B)
    # Stage-2 coef carries the inverse weight scale (psum holds
    # wscale * proj).
    coef32 = np.ascontiguousarray((coef / wscale).astype(np.float32))
    in_maps = []
    for c in range(NCORES):
        wsh = weight[c * OUTL:(c + 1) * OUTL]                # (OUTL, IN, RANK)
        # W2[i, o*RANK+r] = wsh[o, i, r]; n-major 512-col chunks; then swizzle
        # (n, i=k*128+p, c) -> (n, p, k, c) so each partition reads one
        # contiguous 8KB run per n-chunk DMA.
        w2 = wsh.transpose(1, 0, 2).reshape(IN, OUTL * RANK) * wscale
        w2 = w2.reshape(NK, P, NCH, CH).transpose(2, 1, 0, 3)
        w2 = np.ascontiguousarray(w2.reshape(NCH, P, NK * CH)).astype(npwdt)
        biasT = np.ascontiguousarray(
            bias[c * OUTL:(c + 1) * OUTL].T
        ).astype(npdt)                                       # (RANK, OUTL)
        in_maps.append({
            "xT": xT, "w2": w2, "coef": coef32,
            "coefT": coefT, "biasT": biasT,
        })
    return in_maps


_NC_CACHE = {}


def _ensure_ntff_hook():
    """The agent image's antenv lacks axon_hooks; inject it and register
    the ctypes NTFF profile hook so trace=True works under axon."""
    import types
    import antenv
    try:
        from antenv import axon_hooks  # noqa: F401
        return
    except ImportError:
        pass
    mod = types.ModuleType("antenv.axon_hooks")
    _state = {"hook": None}
    mod.set_axon_ntff_profile_hook = lambda h: _state.__setitem__("hook", h)
    mod.get_axon_ntff_profile_hook = lambda: _state["hook"]
    sys.modules["antenv.axon_hooks"] = mod
    antenv.axon_hooks = mod
    try:
        from trn_agent_boot.trn_boot import _ntff_profile_via_ctypes
        mod.set_axon_ntff_profile_hook(
            _ntff_profile_via_ctypes("/opt/axon/libaxon_pjrt.so")
        )
    except Exception:
        pass


def build_nc(dt_name=DT_NAME, impl=None):
    impl = impl or IMPL
    if impl == "raw":
        return build_nc_raw(dt_name)
    if impl == "raw2":
        return build_nc_raw2(dt_name)
    return build_nc_tile(dt_name)


def run(inputs, trace=False, dt_name=DT_NAME, impl=None, **kwargs):
    if trace:
        _ensure_ntff_hook()
    impl = impl or IMPL
    key = (dt_name, impl)
    if key not in _NC_CACHE:
        _NC_CACHE[key] = build_nc(dt_name, impl)
    nc = _NC_CACHE[key]
    in_maps = prepare_in_maps(
        np.asarray(inputs["input"], dtype=np.float32),
        np.asarray(inputs["coef"], dtype=np.float32),
        np.asarray(inputs["weight"], dtype=np.float32),
        np.asarray(inputs["bias"], dtype=np.float32),
        dt_name,
        impl,
    )
    br = run_bass_kernel_spmd(
        nc, in_maps, list(range(NCORES)), trace=trace, **kwargs
    )
    full = np.concatenate(
        [br.results[c]["out"] for c in range(NCORES)], axis=1
    ).astype(np.float32)
    return full, br


def kernel(**inputs):
    full, _ = run(inputs)
    return full

